# revision 4
# baseline (speedup 1.0000x reference)
"""Bi-directional GRU decoder kernel for Trainium2 (8 NeuronCores, SPMD data-parallel).

Problem: B=8192, T=524, D=1, H=32, out K=256.
  gx = x*w_ih^T + b_ih ; GRU scan fwd + bwd (time-reversed); head on concat(h_f, h_b).

Strategy per core (B_local=1024):
  - 4 batch chunks of 256 stacked on partitions: state H_d [128, 256] bf16,
    H_d[32c+k, j] = h_dir[256c+j, k].
  - Per step+dir, gate pre-activations via PSUM-accumulated matmuls:
      P_rz[:,0:256] = blkdiag4(Whh_r^T) @ H  (+)  [x-row blkdiag + bias row] @ XB
      P_rz[:,256:512] same for z ; P_n[:,0:256]=ghn ; P_n[:,256:512]=gxn
    XB [5,256] = 4 x-chunk rows + ones row, DMA'd from a host-prepacked [T,5,256].
  - ACT: sigmoid on P_rz -> RZ; tanh on u -> N (both funcs in one table set).
  - DVE/GpSimd: t=r*ghn ; u=t+gxn ; d=H-N (gpsimd) ; e=z*d ; H=N+e.
  - Head: out^T[256,1024] = [Wout^T; b_out] matmuls on [h_f;h_b;1] per chunk.
"""

import numpy as np

H = 32
B = 8192
T = 524
KOUT = 256
NCORES = 8
BL = B // NCORES  # 1024
NCH = 4
CW = 256  # chunk width

_CACHE = {}


def _build_program(t_steps):
    import concourse.bacc as bacc
    import concourse.mybir as mybir
    from concourse.tile import TileContext
    from concourse.bass import MemorySpace

    bf16 = mybir.dt.bfloat16
    f32 = mybir.dt.float32
    AF = mybir.ActivationFunctionType

    nc = bacc.Bacc()

    xb_h = nc.dram_tensor("xb", [t_steps, 5, CW], bf16, kind="ExternalInput")
    wh_h = nc.dram_tensor("wh", [6, 128, 128], bf16, kind="ExternalInput")
    wx_h = nc.dram_tensor("wx", [8, 5, 128], bf16, kind="ExternalInput")
    wo_h = nc.dram_tensor("wo", [2, 65, 128], bf16, kind="ExternalInput")
    out_h = nc.dram_tensor("outT", [KOUT, BL], f32, kind="ExternalOutput")

    xb = xb_h[:]
    wh = wh_h[:]
    wx = wx_h[:]
    wo = wo_h[:]
    outT = out_h[:]

    with TileContext(nc) as tc:
        with (
            tc.tile_pool(name="consts", bufs=1) as consts,
            tc.tile_pool(name="xbp", bufs=8) as xbp,
            tc.tile_pool(name="psum", bufs=2, space=MemorySpace.PSUM) as psum,
            tc.tile_pool(name="work", bufs=4) as work,
            tc.tile_pool(name="headp", bufs=2) as headp,
        ):
            WH = consts.tile([128, 6 * 128], bf16, name="WH", tag="WH")
            WX = consts.tile([5, 8 * 128], bf16, name="WX", tag="WX")
            WO = consts.tile([65, 2 * 128], bf16, name="WO", tag="WO")
            HS = [
                consts.tile([128, CW], bf16, name=f"Hst{d}", tag=f"Hst{d}")
                for d in range(2)
            ]
            OUT_SB = consts.tile([128, 2048], f32, name="OUT_SB", tag="OUT_SB")

            for k in range(6):
                nc.sync.dma_start(out=WH[:, k * 128:(k + 1) * 128], in_=wh[k])
            for k in range(8):
                nc.sync.dma_start(out=WX[:, k * 128:(k + 1) * 128], in_=wx[k])
            for k in range(2):
                nc.sync.dma_start(out=WO[:, k * 128:(k + 1) * 128], in_=wo[k])
            for d in range(2):
                nc.vector.memset(HS[d][:], 0.0)

            for t in range(t_steps):
                xbt = [None, None]
                for d in range(2):
                    tt = t if d == 0 else (t_steps - 1 - t)
                    xbt[d] = xbp.tile([5, CW], bf16, name=f"XB{d}_{t}", tag=f"XB{d}")
                    nc.sync.dma_start(out=xbt[d][:], in_=xb[tt])

                prz = [None, None]
                pn = [None, None]
                for d in range(2):
                    prz[d] = psum.tile([128, 2 * CW], f32, name=f"prz{d}_{t}", tag=f"prz{d}")
                    pn[d] = psum.tile([128, 2 * CW], f32, name=f"pn{d}_{t}", tag=f"pn{d}")
                    w0 = d * 3 * 128
                    x0 = d * 4 * 128
                    # r gate
                    nc.tensor.matmul(prz[d][:, 0:CW], WH[:, w0:w0 + 128], HS[d][:],
                                     start=True, stop=False)
                    nc.tensor.matmul(prz[d][:, 0:CW], WX[:, x0:x0 + 128], xbt[d][:],
                                     start=False, stop=True)
                    # z gate
                    nc.tensor.matmul(prz[d][:, CW:2 * CW], WH[:, w0 + 128:w0 + 256], HS[d][:],
                                     start=True, stop=False)
                    nc.tensor.matmul(prz[d][:, CW:2 * CW], WX[:, x0 + 128:x0 + 256], xbt[d][:],
                                     start=False, stop=True)
                    # ghn (W@h + b_hh_n)
                    nc.tensor.matmul(pn[d][:, 0:CW], WH[:, w0 + 256:w0 + 384], HS[d][:],
                                     start=True, stop=False)
                    nc.tensor.matmul(pn[d][:, 0:CW], WX[:, x0 + 256:x0 + 384], xbt[d][:],
                                     start=False, stop=True)
                    # gxn (w_ih_n*x + b_ih_n)
                    nc.tensor.matmul(pn[d][:, CW:2 * CW], WX[:, x0 + 384:x0 + 512], xbt[d][:],
                                     start=True, stop=True)

                for d in range(2):
                    RZ = work.tile([128, 2 * CW], bf16, name=f"RZ{d}_{t}", tag=f"RZ{d}")
                    nc.scalar.activation(RZ[:], prz[d][:], AF.Sigmoid)
                    TT = work.tile([128, CW], bf16, name=f"TT{d}_{t}", tag=f"TT{d}")
                    nc.vector.tensor_mul(TT[:], RZ[:, 0:CW], pn[d][:, 0:CW])
                    UU = work.tile([128, CW], bf16, name=f"UU{d}_{t}", tag=f"UU{d}")
                    nc.vector.tensor_add(UU[:], TT[:], pn[d][:, CW:2 * CW])
                    NN = work.tile([128, CW], bf16, name=f"NN{d}_{t}", tag=f"NN{d}")
                    nc.scalar.activation(NN[:], UU[:], AF.Tanh)
                    DD = work.tile([128, CW], bf16, name=f"DD{d}_{t}", tag=f"DD{d}")
                    nc.gpsimd.tensor_sub(DD[:], HS[d][:], NN[:])
                    EE = work.tile([128, CW], bf16, name=f"EE{d}_{t}", tag=f"EE{d}")
                    nc.vector.tensor_mul(EE[:], RZ[:, CW:2 * CW], DD[:])
                    nc.vector.tensor_add(HS[d][:], NN[:], EE[:])

            # ---- head: outT[k, 256c+j] = sum_m wo[k,m]*pooled[256c+j, m] + b_out[k]
            for c in range(NCH):
                hr = headp.tile([65, CW], bf16, name=f"hr_{c}", tag="hr")
                nc.sync.dma_start(out=hr[0:32, :], in_=HS[0][32 * c:32 * c + 32, :])
                nc.sync.dma_start(out=hr[32:64, :], in_=HS[1][32 * c:32 * c + 32, :])
                nc.vector.memset(hr[64:65, :], 1.0)
                for half in range(2):
                    ph = psum.tile([128, 2 * CW], f32, name=f"ph_{c}_{half}", tag="prz0")
                    nc.tensor.matmul(ph[:, 0:CW], WO[:, half * 128:(half + 1) * 128], hr[:],
                                     start=True, stop=True)
                    off = half * 1024 + c * CW
                    nc.scalar.copy(OUT_SB[:, off:off + CW], ph[:, 0:CW])
            for half in range(2):
                nc.sync.dma_start(out=outT[half * 128:(half + 1) * 128, :],
                                  in_=OUT_SB[:, half * 1024:(half + 1) * 1024])

    nc.finalize()
    return nc


def _pack_weights(inputs, bf):
    """Build the blkdiag lhsT matrices (host-side, replicated to all cores)."""
    e4 = np.eye(NCH, dtype=np.float32)

    def blk(w):  # w [32(gate rows g), 32(k)] -> [128(k-chunks), 128(g-chunks)]
        return np.kron(e4, w.T)

    wh = np.zeros((6, 128, 128), np.float32)
    wx = np.zeros((8, 5, 128), np.float32)
    for d, sfx in enumerate(("f", "b")):
        w_ih = np.asarray(inputs[f"w_ih_{sfx}"], np.float32)  # [96, 1]
        w_hh = np.asarray(inputs[f"w_hh_{sfx}"], np.float32)  # [96, 32]
        b_ih = np.asarray(inputs[f"b_ih_{sfx}"], np.float32)  # [96]
        b_hh = np.asarray(inputs[f"b_hh_{sfx}"], np.float32)
        for g in range(3):  # r, z, n
            wh[d * 3 + g] = blk(w_hh[g * H:(g + 1) * H, :])
        # x-type lhsTs: rows 0:4 x-weight blkdiag, row 4 bias
        xr = np.kron(e4, w_ih[0:H, 0].reshape(1, H))          # [4, 128]
        xz = np.kron(e4, w_ih[H:2 * H, 0].reshape(1, H))
        xn = np.kron(e4, w_ih[2 * H:3 * H, 0].reshape(1, H))
        wx[d * 4 + 0, 0:4] = xr
        wx[d * 4 + 0, 4] = np.tile(b_ih[0:H] + b_hh[0:H], NCH)
        wx[d * 4 + 1, 0:4] = xz
        wx[d * 4 + 1, 4] = np.tile(b_ih[H:2 * H] + b_hh[H:2 * H], NCH)
        # ghn bias only (x rows zero)
        wx[d * 4 + 2, 4] = np.tile(b_hh[2 * H:3 * H], NCH)
        wx[d * 4 + 3, 0:4] = xn
        wx[d * 4 + 3, 4] = np.tile(b_ih[2 * H:3 * H], NCH)

    w_out = np.asarray(inputs["w_out"], np.float32)  # [256, 64]
    b_out = np.asarray(inputs["b_out"], np.float32)  # [256]
    wo = np.zeros((2, 65, 128), np.float32)
    for half in range(2):
        wo[half, 0:64] = w_out[half * 128:(half + 1) * 128, :].T
        wo[half, 64] = b_out[half * 128:(half + 1) * 128]

    return wh.astype(bf), wx.astype(bf), wo.astype(bf)


def _pack_xb(inputs, bf):
    x = np.asarray(inputs["x"], np.float32).reshape(B, T)
    xT = np.ascontiguousarray(x.T)  # [T, B]
    xb_all = np.ones((NCORES, T, 5, CW), np.float32)
    for i in range(NCORES):
        xb_all[i, :, 0:4, :] = xT[:, i * BL:(i + 1) * BL].reshape(T, NCH, CW)
    return xb_all.astype(bf)


def kernel(**inputs):
    import ml_dtypes
    from concourse.bass_utils import run_bass_kernel_spmd

    bf = ml_dtypes.bfloat16
    wh, wx, wo = _pack_weights(inputs, bf)
    xb_all = _pack_xb(inputs, bf)

    if T not in _CACHE:
        _CACHE[T] = _build_program(T)
    nc = _CACHE[T]

    in_maps = [
        {"xb": xb_all[i], "wh": wh, "wx": wx, "wo": wo}
        for i in range(NCORES)
    ]
    res = run_bass_kernel_spmd(nc, in_maps, core_ids=list(range(NCORES)))
    outT = np.concatenate([r["outT"] for r in res.results], axis=1)  # [256, 8192]
    return np.ascontiguousarray(outT.T.astype(np.float32))


# revision 21
# speedup vs baseline: 1521.0507x; 1521.0507x over previous
"""Bi-directional GRU decoder kernel for Trainium2 (8 NeuronCores, SPMD data-parallel).

Problem: B=8192, T=524, D=1, H=32, out K=256.
  gx = x*w_ih^T + b_ih ; GRU scan fwd + bwd (time-reversed); head on concat(h_f, h_b).

Strategy per core (B_local=1024):
  - 4 batch chunks of 256 stacked on partitions: state H_d [128, 256] bf16,
    H_d[32c+k, j] = h_dir[256c+j, k].
  - Gate pre-activations via PSUM-accumulated matmuls with block-diagonal
    lhsT = kron(I4, W^T).  h' = s + v is *not* formed before the matmuls:
    W@h' = W@s + W@v (linearity), so the update add is off the critical path.
  - z columns are negated so sigma yields zbar = 1-z directly:
      h' = (h - zbar*h) + zbar*n = s + v.
  - gxn = w_ih_n * x + b_ih_n computed as a per-partition tensor_scalar on a
    replicated-x tile XR (no PSUM operand -> cheap bf16 adds downstream).
  - ACT: sigmoid r-half / sigmoid zbar-half / tanh (all one table set).
  - Engine split: DVE: t, u, gxn, v, h'.  GpSimd: w, s.  PE: 9 matmuls+x per dir.
"""

import numpy as np

H = 32
B = 8192
T = 524
KOUT = 256
NCORES = 8
BL = B // NCORES  # 1024
NCH = 4
CW = 256  # chunk width

_CACHE = {}


def _build_program(t_steps):
    import concourse.bacc as bacc
    import concourse.mybir as mybir
    from concourse.tile import TileContext
    from concourse.bass import MemorySpace

    bf16 = mybir.dt.bfloat16
    f32 = mybir.dt.float32
    AF = mybir.ActivationFunctionType
    OP = mybir.AluOpType

    nc = bacc.Bacc()

    xb_h = nc.dram_tensor("xb", [t_steps, 5, CW], bf16, kind="ExternalInput")
    xr_h = nc.dram_tensor("xr", [t_steps, 128, CW], bf16, kind="ExternalInput")
    wh_h = nc.dram_tensor("wh", [6, 128, 128], bf16, kind="ExternalInput")
    wx_h = nc.dram_tensor("wx", [8, 5, 128], bf16, kind="ExternalInput")
    wnb_h = nc.dram_tensor("wnb", [2, 128, 2], f32, kind="ExternalInput")
    wo_h = nc.dram_tensor("wo", [2, 65, 128], bf16, kind="ExternalInput")
    out_h = nc.dram_tensor("outT", [KOUT, BL], f32, kind="ExternalOutput")

    xb = xb_h[:]
    xr = xr_h[:]
    wh = wh_h[:]
    wx = wx_h[:]
    wnb = wnb_h[:]
    wo = wo_h[:]
    outT = out_h[:]

    with TileContext(nc) as tc:
        with (
            tc.tile_pool(name="consts", bufs=1) as consts,
            tc.tile_pool(name="xbp", bufs=8) as xbp,
            tc.tile_pool(name="xrp", bufs=8) as xrp,
            tc.tile_pool(name="psum", bufs=2, space=MemorySpace.PSUM) as psum,
            tc.tile_pool(name="work", bufs=6) as work,
            tc.tile_pool(name="headp", bufs=2) as headp,
        ):
            WH = consts.tile([128, 6 * 128], bf16, name="WH", tag="WH")
            WX = consts.tile([5, 8 * 128], bf16, name="WX", tag="WX")
            WNB = consts.tile([128, 4], f32, name="WNB", tag="WNB")
            WO = consts.tile([65, 2 * 128], bf16, name="WO", tag="WO")
            HS = [
                consts.tile([128, CW], bf16, name=f"Hst{d}", tag=f"Hst{d}")
                for d in range(2)
            ]
            OUT_SB = consts.tile([128, 2048], f32, name="OUT_SB", tag="OUT_SB")

            for k in range(6):
                nc.sync.dma_start(out=WH[:, k * 128:(k + 1) * 128], in_=wh[k])
            for k in range(8):
                nc.sync.dma_start(out=WX[:, k * 128:(k + 1) * 128], in_=wx[k])
            for k in range(2):
                nc.sync.dma_start(out=WNB[:, k * 2:(k + 1) * 2], in_=wnb[k])
                nc.sync.dma_start(out=WO[:, k * 128:(k + 1) * 128], in_=wo[k])
            for d in range(2):
                nc.vector.memset(HS[d][:], 0.0)

            prevS = [None, None]
            prevV = [None, None]
            for t in range(t_steps):
                xbt = [None, None]
                xrt = [None, None]
                for d in range(2):
                    tt = t if d == 0 else (t_steps - 1 - t)
                    xbt[d] = xbp.tile([5, CW], bf16, name=f"XB{d}_{t}", tag=f"XB{d}")
                    nc.sync.dma_start(out=xbt[d][:], in_=xb[tt])
                    xrt[d] = xrp.tile([128, CW], bf16, name=f"XR{d}_{t}", tag=f"XR{d}")
                    nc.sync.dma_start(out=xrt[d][:], in_=xr[tt])

                GX = [None, None]
                prz = [None, None]
                pn = [None, None]
                RZ = [None, None]
                TT = [None, None]
                UU = [None, None]
                NN = [None, None]
                WW = [None, None]
                SS = [None, None]
                VV = [None, None]
                for d in range(2):
                    GX[d] = work.tile([128, CW], bf16, name=f"GX{d}_{t}", tag=f"GX{d}")
                    nc.gpsimd.tensor_scalar(GX[d][:], xrt[d][:],
                                            WNB[:, 2 * d:2 * d + 1],
                                            WNB[:, 2 * d + 1:2 * d + 2],
                                            OP.mult, OP.add)
                # PSUM layout: P1 = [r-pre | zbar-pre] (one bank), P2 = [ghn]
                # (one bank). Groups within each bank are strictly sequential
                # (hardware requirement). Group-contiguous emission: claiming a
                # PSUM slot too early head-of-line-blocks the PE FIFO on the
                # pool release, so each group is emitted as one run.
                for d in range(2):
                    prz[d] = psum.tile([128, 2 * CW], f32, name=f"prz{d}_{t}", tag=f"prz{d}")
                    pn[d] = psum.tile([128, CW], f32, name=f"pn{d}_{t}", tag=f"pn{d}")
                    w0 = d * 3 * 128
                    x0 = d * 4 * 128
                    nc.tensor.matmul(prz[d][:, 0:CW], WX[:, x0:x0 + 128], xbt[d][:],
                                     start=True, stop=(t == 0))
                    if t > 0:
                        nc.tensor.matmul(prz[d][:, 0:CW], WH[:, w0:w0 + 128],
                                         prevS[d][:], start=False, stop=False)
                        nc.tensor.matmul(prz[d][:, 0:CW], WH[:, w0:w0 + 128],
                                         prevV[d][:], start=False, stop=True)
                for d in range(2):
                    w0 = d * 3 * 128
                    x0 = d * 4 * 128
                    # zbar group in the P1 bank, after the r group closes
                    nc.tensor.matmul(prz[d][:, CW:2 * CW], WX[:, x0 + 128:x0 + 256],
                                     xbt[d][:], start=True, stop=(t == 0))
                    if t > 0:
                        nc.tensor.matmul(prz[d][:, CW:2 * CW], WH[:, w0 + 128:w0 + 256],
                                         prevS[d][:], start=False, stop=False)
                        nc.tensor.matmul(prz[d][:, CW:2 * CW], WH[:, w0 + 128:w0 + 256],
                                         prevV[d][:], start=False, stop=True)
                    # ghn group (P2): nv gates t
                    nc.tensor.matmul(pn[d][:], WX[:, x0 + 256:x0 + 384], xbt[d][:],
                                     start=True, stop=(t == 0))
                    if t > 0:
                        nc.tensor.matmul(pn[d][:], WH[:, w0 + 256:w0 + 384],
                                         prevS[d][:], start=False, stop=False)
                        nc.tensor.matmul(pn[d][:], WH[:, w0 + 256:w0 + 384],
                                         prevV[d][:], start=False, stop=True)
                for d in range(2):
                    # sigma on r-half only: critical path to t
                    RZ[d] = work.tile([128, 2 * CW], bf16, name=f"RZ{d}_{t}", tag=f"RZ{d}")
                    nc.scalar.activation(RZ[d][:, 0:CW], prz[d][:, 0:CW], AF.Sigmoid)
                for d in range(2):
                    TT[d] = work.tile([128, CW], bf16, name=f"TT{d}_{t}", tag=f"TT{d}")
                    nc.vector.tensor_mul(TT[d][:], RZ[d][:, 0:CW], pn[d][:])
                for d in range(2):
                    # zbar = sigmoid(-zpre) = 1 - z (z columns negated host-side)
                    nc.scalar.activation(RZ[d][:, CW:2 * CW], prz[d][:, CW:2 * CW], AF.Sigmoid)
                for d in range(2):
                    UU[d] = work.tile([128, CW], bf16, name=f"UU{d}_{t}", tag=f"UU{d}")
                    nc.vector.tensor_add(UU[d][:], TT[d][:], GX[d][:])
                for d in range(2):
                    # off-critical-path: w = zbar*h ; s = h - w  (gpsimd)
                    WW[d] = work.tile([128, CW], bf16, name=f"WW{d}_{t}", tag=f"WW{d}")
                    nc.gpsimd.tensor_mul(WW[d][:], RZ[d][:, CW:2 * CW], HS[d][:])
                for d in range(2):
                    NN[d] = work.tile([128, CW], bf16, name=f"NN{d}_{t}", tag=f"NN{d}")
                    nc.scalar.activation(NN[d][:], UU[d][:], AF.Tanh)
                for d in range(2):
                    SS[d] = work.tile([128, CW], bf16, name=f"SS{d}_{t}", tag=f"SS{d}")
                    nc.gpsimd.tensor_sub(SS[d][:], HS[d][:], WW[d][:])
                for d in range(2):
                    VV[d] = work.tile([128, CW], bf16, name=f"VV{d}_{t}", tag=f"VV{d}")
                    nc.vector.tensor_mul(VV[d][:], RZ[d][:, CW:2 * CW], NN[d][:])
                for d in range(2):
                    nc.vector.tensor_add(HS[d][:], SS[d][:], VV[d][:])
                prevS = SS
                prevV = VV

            # ---- head: outT[k, 256c+j] = sum_m wo[k,m]*pooled[256c+j, m] + b_out[k]
            for c in range(NCH):
                hr = headp.tile([65, CW], bf16, name=f"hr_{c}", tag="hr")
                nc.sync.dma_start(out=hr[0:32, :], in_=HS[0][32 * c:32 * c + 32, :])
                nc.sync.dma_start(out=hr[32:64, :], in_=HS[1][32 * c:32 * c + 32, :])
                nc.vector.memset(hr[64:65, :], 1.0)
                for half in range(2):
                    ph = psum.tile([128, 2 * CW], f32, name=f"ph_{c}_{half}", tag="prz0")
                    nc.tensor.matmul(ph[:, 0:CW], WO[:, half * 128:(half + 1) * 128], hr[:],
                                     start=True, stop=True)
                    off = half * 1024 + c * CW
                    nc.scalar.copy(OUT_SB[:, off:off + CW], ph[:, 0:CW])
            for half in range(2):
                nc.sync.dma_start(out=outT[half * 128:(half + 1) * 128, :],
                                  in_=OUT_SB[:, half * 1024:(half + 1) * 1024])

    nc.finalize()
    return nc


def _pack_weights(inputs, bf):
    """Build the blkdiag lhsT matrices (host-side, replicated to all cores)."""
    e4 = np.eye(NCH, dtype=np.float32)

    def blk(w):  # w [32(gate rows g), 32(k)] -> [128(k-chunks), 128(g-chunks)]
        return np.kron(e4, w.T)

    wh = np.zeros((6, 128, 128), np.float32)
    wx = np.zeros((8, 5, 128), np.float32)
    wnb = np.zeros((2, 128, 2), np.float32)
    for d, sfx in enumerate(("f", "b")):
        w_ih = np.asarray(inputs[f"w_ih_{sfx}"], np.float32)  # [96, 1]
        w_hh = np.asarray(inputs[f"w_hh_{sfx}"], np.float32)  # [96, 32]
        b_ih = np.asarray(inputs[f"b_ih_{sfx}"], np.float32)  # [96]
        b_hh = np.asarray(inputs[f"b_hh_{sfx}"], np.float32)
        for g in range(3):  # r, z, n
            wh[d * 3 + g] = blk(w_hh[g * H:(g + 1) * H, :])
        wh[d * 3 + 1] *= -1.0  # z columns negated: sigma gives zbar = 1-z
        xr_w = np.kron(e4, w_ih[0:H, 0].reshape(1, H))          # [4, 128]
        xz_w = np.kron(e4, w_ih[H:2 * H, 0].reshape(1, H))
        wx[d * 4 + 0, 0:4] = xr_w
        wx[d * 4 + 0, 4] = np.tile(b_ih[0:H] + b_hh[0:H], NCH)
        wx[d * 4 + 1, 0:4] = -xz_w
        wx[d * 4 + 1, 4] = -np.tile(b_ih[H:2 * H] + b_hh[H:2 * H], NCH)
        # ghn bias only (x rows zero)
        wx[d * 4 + 2, 4] = np.tile(b_hh[2 * H:3 * H], NCH)
        # per-partition scalars for gxn tensor_scalar
        wnb[d, :, 0] = np.tile(w_ih[2 * H:3 * H, 0], NCH)
        wnb[d, :, 1] = np.tile(b_ih[2 * H:3 * H], NCH)

    w_out = np.asarray(inputs["w_out"], np.float32)  # [256, 64]
    b_out = np.asarray(inputs["b_out"], np.float32)  # [256]
    wo = np.zeros((2, 65, 128), np.float32)
    for half in range(2):
        wo[half, 0:64] = w_out[half * 128:(half + 1) * 128, :].T
        wo[half, 64] = b_out[half * 128:(half + 1) * 128]

    return wh.astype(bf), wx.astype(bf), wnb, wo.astype(bf)


def _pack_xb(inputs, bf):
    x = np.asarray(inputs["x"], np.float32).reshape(B, T)
    xT = np.ascontiguousarray(x.T)  # [T, B]
    xb_all = np.ones((NCORES, T, 5, CW), np.float32)
    for i in range(NCORES):
        xb_all[i, :, 0:4, :] = xT[:, i * BL:(i + 1) * BL].reshape(T, NCH, CW)
    xb_all = xb_all.astype(bf)
    # replicated-x tiles: xr[t, 32c+k, j] = x[t, 256c+j]
    xr_all = np.broadcast_to(
        xb_all[:, :, 0:4, :].reshape(NCORES, T, NCH, 1, CW),
        (NCORES, T, NCH, 32, CW),
    ).reshape(NCORES, T, 128, CW)
    return xb_all, np.ascontiguousarray(xr_all)


def kernel(**inputs):
    import ml_dtypes
    from concourse.bass_utils import run_bass_kernel_spmd

    bf = ml_dtypes.bfloat16
    wh, wx, wnb, wo = _pack_weights(inputs, bf)
    xb_all, xr_all = _pack_xb(inputs, bf)

    if T not in _CACHE:
        _CACHE[T] = _build_program(T)
    nc = _CACHE[T]

    in_maps = [
        {"xb": xb_all[i], "xr": xr_all[i], "wh": wh, "wx": wx, "wnb": wnb, "wo": wo}
        for i in range(NCORES)
    ]
    res = run_bass_kernel_spmd(nc, in_maps, core_ids=list(range(NCORES)))
    outT = np.concatenate([r["outT"] for r in res.results], axis=1)  # [256, 8192]
    return np.ascontiguousarray(outT.T.astype(np.float32))
